# revision 39
# baseline (speedup 1.0000x reference)
"""Single-head causal attention on 8 Trainium2 NeuronCores.

Problem: x[8, 4096, 384], Wq/Wk/Wv[384, 64] ->
    out[b] = softmax(causal((x[b]Wq)(x[b]Wk)^T / sqrt(384))) @ (x[b]Wv)

Sharding: data-parallel over batch — core i computes batch element i.
Weights are replicated to every core.

Per-core kernel layout (all matmuls contract over the partition axis):
  - X^T tiles [c=128, t] are built from natural x tiles via PE transposes.
  - Q^T, K^T [64, T] are computed together, column-packed: one PSUM tile
    [128, 512] holds Q^T in partitions 0:64 and K^T in 64:128; the two
    matmuls per embed chunk run concurrently in disjoint column groups.
    Q^T is stored twice (partitions 0:64 and 64:128) so score matmuls
    can be row-packed two-at-a-time into the 128x128 PE array.
  - V_ext [t=128, 65] = [X Wv | 1]  (ones column -> softmax denominator).
  - Scores are computed TRANSPOSED: S^T[s, q] = K Q^T so that the
    softmax sum over s becomes a matmul-friendly partition axis and
    P^T tiles feed the PV matmul with no per-tile transposes:
        O^T[h+1, q] += V_ext[s,:]^T @ P^T[s, q]   (row 64 = sum_s P)
  - exp via ScalarE activation (no max subtraction: |scores/sqrt(C)| is
    small for this distribution, exp cannot overflow in fp32).
  - Causal masking: multiply diagonal-block P^T tiles by one of four
    precomputed 0/1 masks (built once with gpsimd.affine_select).
  - O^T goes back to natural layout via the DMA XBAR transpose
    ([80, 512] fp16 -> [128, 4, 80]); column 64 holds the row sums;
    divide and DMA out.
  - Prologue: blocks 0-1 load x as fp32 on the fast HWDGE queues
    (sync+scalar) and cast on DVE — the gpsimd casting DMA is ~5x
    slower and would put ~10us on the startup critical path.  The exp
    ACT table is preloaded with a dummy activation.
"""

import sys

if "/opt/trn_rl_repo" not in sys.path:
    sys.path.insert(0, "/opt/trn_rl_repo")

import numpy as np

import concourse.bass as bass  # noqa: F401  (AP types used implicitly)
import concourse.tile as tile
from concourse import bacc, mybir
from concourse.bass import ds
from concourse.bass_utils import run_bass_kernel_spmd
from concourse.masks import make_identity

B = 8
T_FULL = 4096
C = 384
H = 64
P = 128
TQ = 512  # q-block width
VW = 80  # padded O^T row count (mult of 16 for the XBAR transpose)
SCALE = 1.0 / float(np.sqrt(C))
F32 = mybir.dt.float32

F16 = mybir.dt.float16
MM_DTYPE = F16  # matmul pipeline dtype (fp16: 1 cyc/row + fast weight load)
N_FAST = 4  # blocks loaded via fast fp32 HWDGE path at startup


def build_nc(T=T_FULL, mm_dtype=MM_DTYPE):
    """Build the per-core Bass program (same program on all 8 cores)."""
    NT = T // P  # number of 128-row s-chunks
    NQ = T // TQ  # number of 512-row q-blocks
    CC = C // P  # 3 embed chunks
    SUB = TQ // P  # 4 sub-tiles per block

    MMD = mm_dtype  # tiles feeding matmuls are allocated in this dtype

    nc = bacc.Bacc(
        "TRN2",
        target_bir_lowering=False,
        debug=False,
        enable_asserts=True,
        num_devices=B,
    )
    x_ap = nc.dram_tensor("x", [T, C], F32, kind="ExternalInput").ap()
    wq_ap = nc.dram_tensor("Wq", [C, H], F32, kind="ExternalInput").ap()
    wk_ap = nc.dram_tensor("Wk", [C, H], F32, kind="ExternalInput").ap()
    wv_ap = nc.dram_tensor("Wv", [C, H], F32, kind="ExternalInput").ap()
    out_ap = nc.dram_tensor("out", [T, H], F32, kind="ExternalOutput").ap()

    x_re = x_ap.rearrange("(n p) c -> p n c", p=P)  # [128, NT, 384]
    out_re = out_ap.rearrange("(n p) h -> p n h", p=P)  # [128, NT, 64]

    with tile.TileContext(nc) as tc:
        with (
            tc.tile_pool(name="consts", bufs=1) as consts,
            tc.tile_pool(name="xnat", bufs=8) as xnat,
            tc.tile_pool(name="xtp", bufs=3) as xtp,
            tc.tile_pool(name="qkt", bufs=1) as qktp,
            tc.tile_pool(name="vextp", bufs=1) as vextp,
            tc.tile_pool(name="ptp", bufs=6) as ptp,
            tc.tile_pool(name="otp", bufs=2) as otp,
            tc.tile_pool(name="vtp", bufs=2) as vtp,
            tc.tile_pool(name="op", bufs=2) as op_,
            tc.tile_pool(name="rvp", bufs=2) as rvp,
            tc.tile_pool(name="psum", bufs=2, space="PSUM") as psum,
        ):
            # fast fp32 loads for the first blocks go FIRST so nothing
            # delays the load -> transpose -> QK -> scores -> exp chain
            # (the gpsimd casting DMA is ~5x slower; later blocks use it
            # because their latency is hidden by the attention loop).
            xn_pre = {}
            for j in range(N_FAST):
                xn32 = xnat.tile([P, SUB, C], F32, tag="xn32", name=f"xn32_{j}")
                h0 = SUB * j
                nc.sync.dma_start(
                    out=xn32[:, 0 : SUB // 2, :], in_=x_re[:, h0 : h0 + SUB // 2, :]
                )
                eng = nc.scalar if j < 2 else nc.gpsimd
                eng.dma_start(
                    out=xn32[:, SUB // 2 : SUB, :],
                    in_=x_re[:, h0 + SUB // 2 : h0 + SUB, :],
                )
                xn_pre[j] = xn32

            ident_h = consts.tile([P, P], MMD)
            make_identity(nc, ident_h)
            # preload the exp ACT table during the prologue
            exp_warm = consts.tile([1, 1], F32)
            nc.vector.memset(exp_warm, 0.0)
            nc.scalar.activation(
                out=exp_warm, in_=exp_warm, func=mybir.ActivationFunctionType.Exp
            )
            wq_sb = consts.tile([P, CC, H], MMD)
            nc.gpsimd.dma_start(out=wq_sb, in_=wq_ap.rearrange("(c p) h -> p c h", p=P))
            wk_sb = consts.tile([P, CC, H], MMD)
            nc.gpsimd.dma_start(out=wk_sb, in_=wk_ap.rearrange("(c p) h -> p c h", p=P))
            wv_sb = consts.tile([P, CC, H], MMD)
            nc.gpsimd.dma_start(out=wv_sb, in_=wv_ap.rearrange("(c p) h -> p c h", p=P))

            # masks[d][s_local, q_local] = 1.0 where q_local - s_local - 128*d >= 0
            masks = consts.tile([P, SUB, TQ], MMD)
            nc.vector.memset(masks, 1.0)
            for d in range(SUB):
                nc.gpsimd.affine_select(
                    out=masks[:, d, :],
                    in_=masks[:, d, :],
                    compare_op=mybir.AluOpType.is_ge,
                    fill=0.0,
                    base=-P * d,
                    pattern=[[1, TQ]],
                    channel_multiplier=-1,
                )

            # qt2: Q^T duplicated in both partition halves.
            # kt2: K^T chunk c lives at partitions 64*(c%2), col (c//2)*128.
            qt2 = qktp.tile([P, T], MMD, tag="qt")
            kt2 = qktp.tile([P, (NT // 2) * P], MMD, tag="kt")
            vext = vextp.tile([P, NT, VW], MMD)
            ones_col = consts.tile([P, NT, 1], F32)
            nc.vector.memset(ones_col, 1.0)
            nc.vector.tensor_copy(out=vext[:, :, H : H + 1], in_=ones_col)

            def phase1_gen(j):
                """Load x rows [512j, 512j+512), produce X^T, Q^T, K^T, V.

                Yields between small PE chunks so the driver can spread
                this work into the gaps of the ScalarE-bound attention
                pair loop without ever blocking the pss->exp pipeline
                (phase-1 PSUM lives on the "acc" tag, not "wide").
                """
                if j in xn_pre:
                    xn = xnat.tile([P, SUB, C], MMD, tag="xn", name=f"xn{j}")
                    xn32 = xn_pre.pop(j)
                    nc.vector.tensor_copy(
                        out=xn[:, 0 : SUB // 2, :], in_=xn32[:, 0 : SUB // 2, :]
                    )
                    nc.vector.tensor_copy(
                        out=xn[:, SUB // 2 : SUB, :], in_=xn32[:, SUB // 2 : SUB, :]
                    )
                else:
                    xn = xnat.tile([P, SUB, C], MMD, tag="xn", name=f"xn{j}")
                    nc.gpsimd.dma_start(
                        out=xn, in_=x_re[:, SUB * j : SUB * (j + 1), :]
                    )
                xt = xtp.tile([P, CC, TQ], MMD, tag="xt", name=f"xt{j}")
                yield
                for st in range(SUB):
                    pst = psum.tile([P, CC, P], MMD, tag="small", name=f"pst{j}_{st}")
                    for c in range(CC):
                        nc.tensor.transpose(
                            pst[:, c, :], xn[:, st, c * P : (c + 1) * P], ident_h
                        )
                    nc.vector.tensor_copy(
                        out=xt[:, :, st * P : (st + 1) * P], in_=pst
                    )
                    yield
                blk = ds(j * TQ, TQ)
                # Q^T and K^T column-packed into one PSUM tile: Q^T in
                # partitions 0:64 (col groups 0-1), K^T in 64:128 (2-3).
                psqk = psum.tile([P, TQ], F32, tag="acc", name=f"psqk{j}")
                for c in range(CC):
                    nc.tensor.matmul(
                        psqk[0:64, :],
                        lhsT=wq_sb[:, c, :],
                        rhs=xt[:, c, :],
                        start=(c == 0),
                        stop=(c == CC - 1),
                        tile_position=(0, 0),
                    )
                    nc.tensor.matmul(
                        psqk[64:128, :],
                        lhsT=wk_sb[:, c, :],
                        rhs=xt[:, c, :],
                        start=(c == 0),
                        stop=(c == CC - 1),
                        tile_position=(0, 64),
                    )
                nc.vector.tensor_copy(out=qt2[0:H, blk], in_=psqk[0:64, :])
                nc.vector.tensor_copy(out=qt2[H:P, blk], in_=psqk[0:64, :])
                yield
                for st in range(SUB):
                    c = SUB * j + st
                    half = H * (c % 2)
                    nc.vector.tensor_copy(
                        out=kt2[half : half + H, (c // 2) * P : (c // 2 + 1) * P],
                        in_=psqk[64:128, st * P : (st + 1) * P],
                    )
                yield
                psv = psum.tile([H, TQ], F32, tag="acc", name=f"psv{j}")
                for c in range(CC):
                    nc.tensor.matmul(
                        psv,
                        lhsT=wv_sb[:, c, :],
                        rhs=xt[:, c, :],
                        start=(c == 0),
                        stop=(c == CC - 1),
                    )
                vt = vtp.tile([H, TQ], MMD, tag="vt", name=f"vt{j}")
                nc.vector.tensor_copy(out=vt, in_=psv)
                yield
                # V natural via the DMA XBAR: [64, 512] -> [128, 4, 64]
                # (not latency-critical: produced two blocks ahead of use)
                nc.sync.dma_start_transpose(
                    out=vext[:, SUB * j : SUB * (j + 1), 0:H], in_=vt
                )
                yield

            N1_CHUNKS = 9

            def phase2(j, pump):
                """Attention for q rows [512j, 512j+512).  pump(done, total)
                advances the interleaved next-block phase-1 generator."""
                nchunks = (j + 1) * SUB
                q_sl = ds(j * TQ, TQ)
                npairs = nchunks // 2
                pso = psum.tile([H + 1, TQ], F32, tag="acc", name=f"pso{j}")
                for pr in range(npairs):
                    pss = psum.tile([P, 2 * TQ], F32, tag="wide", name=f"pss{j}_{pr}")
                    for h2 in range(2):
                        c = 2 * pr + h2
                        half = H * (c % 2)
                        nc.tensor.matmul(
                            pss[:, h2 * TQ : (h2 + 1) * TQ],
                            lhsT=kt2[half : half + H, (c // 2) * P : (c // 2 + 1) * P],
                            rhs=qt2[half : half + H, q_sl],
                            start=True,
                            stop=True,
                            tile_position=(half, 0),
                        )
                    pt = ptp.tile([P, 2 * TQ], MMD, tag="pt", name=f"pt{j}_{pr}")
                    nc.scalar.activation(
                        out=pt,
                        in_=pss,
                        func=mybir.ActivationFunctionType.Exp,
                        scale=SCALE,
                    )
                    for h2 in range(2):
                        c = 2 * pr + h2
                        d = c - SUB * j
                        if d >= 0:
                            nc.vector.tensor_mul(
                                out=pt[:, h2 * TQ : (h2 + 1) * TQ],
                                in0=pt[:, h2 * TQ : (h2 + 1) * TQ],
                                in1=masks[:, d, :],
                            )
                    for h2 in range(2):
                        c = 2 * pr + h2
                        nc.tensor.matmul(
                            pso,
                            lhsT=vext[:, c, 0 : H + 1],
                            rhs=pt[:, h2 * TQ : (h2 + 1) * TQ],
                            start=(c == 0),
                            stop=(c == nchunks - 1),
                        )
                    pump(pr + 1, npairs)
                ot = otp.tile([VW, TQ], MMD, tag="ot", name=f"ot{j}")
                nc.vector.tensor_copy(out=ot[0 : H + 1, :], in_=pso)
                # O^T -> O via the DMA XBAR: [80, 512] -> [128, 4, 80]
                o16 = op_.tile([P, SUB, VW], MMD, tag="o16", name=f"o16{j}")
                nc.sync.dma_start_transpose(out=o16, in_=ot)
                rv = rvp.tile([P, SUB], F32, tag="rv", name=f"rv{j}")
                nc.vector.reciprocal(out=rv, in_=o16[:, :, H : H + 1])
                o = op_.tile([P, SUB, H], F32, tag="o", name=f"o{j}")
                for i in range(SUB):
                    nc.vector.tensor_scalar_mul(
                        out=o[:, i, :],
                        in0=o16[:, i, 0:H],
                        scalar1=rv[:, i : i + 1],
                    )
                nc.gpsimd.dma_start(
                    out=out_re[:, SUB * j : SUB * (j + 1), :], in_=o
                )

            for j in range(min(2, NQ)):
                for _ in phase1_gen(j):
                    pass
            for j in range(NQ):
                gen = phase1_gen(j + 2) if j + 2 < NQ else None
                adv = {"n": 0}

                def pump(done, total, gen=gen, adv=adv):
                    if gen is None:
                        return
                    want = done * N1_CHUNKS // total
                    while adv["n"] < want:
                        try:
                            next(gen)
                        except StopIteration:
                            break
                        adv["n"] += 1

                phase2(j, pump)
                if gen is not None:
                    for _ in gen:
                        pass

    nc.compile()
    return nc


_NC_CACHE = {}


def _get_nc():
    if "nc" not in _NC_CACHE:
        _NC_CACHE["nc"] = build_nc()
    return _NC_CACHE["nc"]


def kernel(x, Wk, Wq, Wv, _trace=False, _trace_kwargs=None):
    x = np.ascontiguousarray(x, dtype=np.float32)
    Wk = np.ascontiguousarray(Wk, dtype=np.float32)
    Wq = np.ascontiguousarray(Wq, dtype=np.float32)
    Wv = np.ascontiguousarray(Wv, dtype=np.float32)
    nc = _get_nc()
    in_maps = [
        {"x": x[b], "Wq": Wq, "Wk": Wk, "Wv": Wv} for b in range(B)
    ]
    res = run_bass_kernel_spmd(
        nc, in_maps, list(range(B)), trace=_trace, **(_trace_kwargs or {})
    )
    out = np.stack([res.results[b]["out"] for b in range(B)], axis=0)
    if _trace:
        return out, res
    return out


# revision 41
# speedup vs baseline: 1.0206x; 1.0206x over previous
"""Single-head causal attention on 8 Trainium2 NeuronCores.

Problem: x[8, 4096, 384], Wq/Wk/Wv[384, 64] ->
    out[b] = softmax(causal((x[b]Wq)(x[b]Wk)^T / sqrt(384))) @ (x[b]Wv)

Sharding: data-parallel over batch — core i computes batch element i.
Weights are replicated to every core.

Per-core kernel layout (all matmuls contract over the partition axis):
  - X^T tiles [c=128, t] are built from natural x tiles via PE transposes.
  - Q^T, K^T [64, T] are computed together, column-packed: one PSUM tile
    [128, 512] holds Q^T in partitions 0:64 and K^T in 64:128; the two
    matmuls per embed chunk run concurrently in disjoint column groups.
    Q^T is stored twice (partitions 0:64 and 64:128) so score matmuls
    can be row-packed two-at-a-time into the 128x128 PE array.
  - V_ext [t=128, 65] = [X Wv | 1]  (ones column -> softmax denominator).
  - Scores are computed TRANSPOSED: S^T[s, q] = K Q^T so that the
    softmax sum over s becomes a matmul-friendly partition axis and
    P^T tiles feed the PV matmul with no per-tile transposes:
        O^T[h+1, q] += V_ext[s,:]^T @ P^T[s, q]   (row 64 = sum_s P)
  - exp via ScalarE activation (no max subtraction: |scores/sqrt(C)| is
    small for this distribution, exp cannot overflow in fp32).
  - Causal masking: multiply diagonal-block P^T tiles by one of four
    precomputed 0/1 masks (built once with gpsimd.affine_select).
  - O^T goes back to natural layout via the DMA XBAR transpose
    ([80, 512] fp16 -> [128, 4, 80]); column 64 holds the row sums;
    divide and DMA out.
  - Prologue: blocks 0-1 load x as fp32 on the fast HWDGE queues
    (sync+scalar) and cast on DVE — the gpsimd casting DMA is ~5x
    slower and would put ~10us on the startup critical path.  The exp
    ACT table is preloaded with a dummy activation.
"""

import sys

if "/opt/trn_rl_repo" not in sys.path:
    sys.path.insert(0, "/opt/trn_rl_repo")

import numpy as np

import concourse.bass as bass  # noqa: F401  (AP types used implicitly)
import concourse.tile as tile
from concourse import bacc, mybir
from concourse.bass import ds
from concourse.bass_utils import run_bass_kernel_spmd
from concourse.masks import make_identity

B = 8
T_FULL = 4096
C = 384
H = 64
P = 128
TQ = 512  # q-block width
VW = 80  # padded O^T row count (mult of 16 for the XBAR transpose)
SCALE = 1.0 / float(np.sqrt(C))
F32 = mybir.dt.float32

F16 = mybir.dt.float16
MM_DTYPE = F16  # matmul pipeline dtype (fp16: 1 cyc/row + fast weight load)
N_FAST = 4  # blocks loaded via fast fp32 HWDGE path at startup


def build_nc(T=T_FULL, mm_dtype=MM_DTYPE):
    """Build the per-core Bass program (same program on all 8 cores)."""
    NT = T // P  # number of 128-row s-chunks
    NQ = T // TQ  # number of 512-row q-blocks
    CC = C // P  # 3 embed chunks
    SUB = TQ // P  # 4 sub-tiles per block

    MMD = mm_dtype  # tiles feeding matmuls are allocated in this dtype

    nc = bacc.Bacc(
        "TRN2",
        target_bir_lowering=False,
        debug=False,
        enable_asserts=True,
        num_devices=B,
    )
    x_ap = nc.dram_tensor("x", [T, C], F32, kind="ExternalInput").ap()
    wq_ap = nc.dram_tensor("Wq", [C, H], F32, kind="ExternalInput").ap()
    wk_ap = nc.dram_tensor("Wk", [C, H], F32, kind="ExternalInput").ap()
    wv_ap = nc.dram_tensor("Wv", [C, H], F32, kind="ExternalInput").ap()
    out_ap = nc.dram_tensor("out", [T, H], F32, kind="ExternalOutput").ap()

    x_re = x_ap.rearrange("(n p) c -> p n c", p=P)  # [128, NT, 384]
    out_re = out_ap.rearrange("(n p) h -> p n h", p=P)  # [128, NT, 64]

    with tile.TileContext(nc) as tc:
        with (
            tc.tile_pool(name="consts", bufs=1) as consts,
            tc.tile_pool(name="xnat", bufs=8) as xnat,
            tc.tile_pool(name="xtp", bufs=3) as xtp,
            tc.tile_pool(name="qkt", bufs=1) as qktp,
            tc.tile_pool(name="vextp", bufs=1) as vextp,
            tc.tile_pool(name="ptp", bufs=6) as ptp,
            tc.tile_pool(name="otp", bufs=2) as otp,
            tc.tile_pool(name="vtp", bufs=2) as vtp,
            tc.tile_pool(name="op", bufs=2) as op_,
            tc.tile_pool(name="rvp", bufs=2) as rvp,
            tc.tile_pool(name="psum", bufs=2, space="PSUM") as psum,
        ):
            # fast fp32 loads for the first blocks go FIRST so nothing
            # delays the load -> transpose -> QK -> scores -> exp chain
            # (the gpsimd casting DMA is ~5x slower; later blocks use it
            # because their latency is hidden by the attention loop).
            xn_pre = {}
            for j in range(N_FAST):
                xn32 = xnat.tile([P, SUB, C], F32, tag="xn32", name=f"xn32_{j}")
                h0 = SUB * j
                nc.sync.dma_start(
                    out=xn32[:, 0 : SUB // 2, :], in_=x_re[:, h0 : h0 + SUB // 2, :]
                )
                eng = nc.scalar if j < 2 else nc.gpsimd
                eng.dma_start(
                    out=xn32[:, SUB // 2 : SUB, :],
                    in_=x_re[:, h0 + SUB // 2 : h0 + SUB, :],
                )
                xn_pre[j] = xn32

            ident_h = consts.tile([P, P], MMD)
            make_identity(nc, ident_h)
            # preload the exp ACT table during the prologue
            exp_warm = consts.tile([1, 1], F32)
            nc.vector.memset(exp_warm, 0.0)
            nc.scalar.activation(
                out=exp_warm, in_=exp_warm, func=mybir.ActivationFunctionType.Exp
            )
            wq_sb = consts.tile([P, CC, H], MMD)
            nc.gpsimd.dma_start(out=wq_sb, in_=wq_ap.rearrange("(c p) h -> p c h", p=P))
            wk_sb = consts.tile([P, CC, H], MMD)
            nc.gpsimd.dma_start(out=wk_sb, in_=wk_ap.rearrange("(c p) h -> p c h", p=P))
            wv_sb = consts.tile([P, CC, H], MMD)
            nc.gpsimd.dma_start(out=wv_sb, in_=wv_ap.rearrange("(c p) h -> p c h", p=P))

            # masks[d][s_local, q_local] = 1.0 where q_local - s_local - 128*d >= 0
            masks = consts.tile([P, SUB, TQ], MMD)
            nc.vector.memset(masks, 1.0)
            for d in range(SUB):
                nc.gpsimd.affine_select(
                    out=masks[:, d, :],
                    in_=masks[:, d, :],
                    compare_op=mybir.AluOpType.is_ge,
                    fill=0.0,
                    base=-P * d,
                    pattern=[[1, TQ]],
                    channel_multiplier=-1,
                )

            # qt2: Q^T duplicated in both partition halves.
            # kt2: K^T chunk c lives at partitions 64*(c%2), col (c//2)*128.
            qt2 = qktp.tile([P, T], MMD, tag="qt")
            kt2 = qktp.tile([P, (NT // 2) * P], MMD, tag="kt")
            vext = vextp.tile([P, NT, VW], MMD)
            ones_col = consts.tile([P, NT, 1], F32)
            nc.vector.memset(ones_col, 1.0)
            nc.vector.tensor_copy(out=vext[:, :, H : H + 1], in_=ones_col)

            def phase1_gen(j):
                """Load x rows [512j, 512j+512), produce X^T, Q^T, K^T, V.

                Yields between small PE chunks so the driver can spread
                this work into the gaps of the ScalarE-bound attention
                pair loop without ever blocking the pss->exp pipeline
                (phase-1 PSUM lives on the "acc" tag, not "wide").
                """
                if j in xn_pre:
                    xn = xnat.tile([P, SUB, C], MMD, tag="xn", name=f"xn{j}")
                    xn32 = xn_pre.pop(j)
                    nc.vector.tensor_copy(
                        out=xn[:, 0 : SUB // 2, :], in_=xn32[:, 0 : SUB // 2, :]
                    )
                    nc.vector.tensor_copy(
                        out=xn[:, SUB // 2 : SUB, :], in_=xn32[:, SUB // 2 : SUB, :]
                    )
                else:
                    xn = xnat.tile([P, SUB, C], MMD, tag="xn", name=f"xn{j}")
                    nc.gpsimd.dma_start(
                        out=xn, in_=x_re[:, SUB * j : SUB * (j + 1), :]
                    )
                xt = xtp.tile([P, CC, TQ], MMD, tag="xt", name=f"xt{j}")
                yield
                for st in range(SUB):
                    pst = psum.tile([P, CC, P], MMD, tag="small", name=f"pst{j}_{st}")
                    for c in range(CC):
                        nc.tensor.transpose(
                            pst[:, c, :], xn[:, st, c * P : (c + 1) * P], ident_h
                        )
                    nc.vector.tensor_copy(
                        out=xt[:, :, st * P : (st + 1) * P], in_=pst
                    )
                    yield
                blk = ds(j * TQ, TQ)
                # Q^T and K^T column-packed into one PSUM tile: Q^T in
                # partitions 0:64 (col groups 0-1), K^T in 64:128 (2-3).
                psqk = psum.tile([P, TQ], F32, tag="acc", name=f"psqk{j}")
                for c in range(CC):
                    nc.tensor.matmul(
                        psqk[0:64, :],
                        lhsT=wq_sb[:, c, :],
                        rhs=xt[:, c, :],
                        start=(c == 0),
                        stop=(c == CC - 1),
                        tile_position=(0, 0),
                    )
                    nc.tensor.matmul(
                        psqk[64:128, :],
                        lhsT=wk_sb[:, c, :],
                        rhs=xt[:, c, :],
                        start=(c == 0),
                        stop=(c == CC - 1),
                        tile_position=(0, 64),
                    )
                nc.vector.tensor_copy(out=qt2[0:H, blk], in_=psqk[0:64, :])
                nc.vector.tensor_copy(out=qt2[H:P, blk], in_=psqk[0:64, :])
                yield
                for st in range(SUB):
                    c = SUB * j + st
                    half = H * (c % 2)
                    nc.vector.tensor_copy(
                        out=kt2[half : half + H, (c // 2) * P : (c // 2 + 1) * P],
                        in_=psqk[64:128, st * P : (st + 1) * P],
                    )
                yield
                psv = psum.tile([H, TQ], F32, tag="acc", name=f"psv{j}")
                for c in range(CC):
                    nc.tensor.matmul(
                        psv,
                        lhsT=wv_sb[:, c, :],
                        rhs=xt[:, c, :],
                        start=(c == 0),
                        stop=(c == CC - 1),
                    )
                vt = vtp.tile([H, TQ], MMD, tag="vt", name=f"vt{j}")
                nc.vector.tensor_copy(out=vt, in_=psv)
                yield
                # V natural via the DMA XBAR: [64, 512] -> [128, 4, 64]
                # (not latency-critical: produced two blocks ahead of use)
                nc.sync.dma_start_transpose(
                    out=vext[:, SUB * j : SUB * (j + 1), 0:H], in_=vt
                )
                yield

            N1_CHUNKS = 9

            def phase2(j, pump):
                """Attention for q rows [512j, 512j+512).  pump(done, total)
                advances the interleaved next-block phase-1 generator."""
                nchunks = (j + 1) * SUB
                q_sl = ds(j * TQ, TQ)
                npairs = nchunks // 2
                pso = psum.tile([H + 1, TQ], F32, tag="acc", name=f"pso{j}")
                for pr in range(npairs):
                    pss = psum.tile([P, 2 * TQ], F32, tag="wide", name=f"pss{j}_{pr}")
                    for h2 in range(2):
                        c = 2 * pr + h2
                        half = H * (c % 2)
                        nc.tensor.matmul(
                            pss[:, h2 * TQ : (h2 + 1) * TQ],
                            lhsT=kt2[half : half + H, (c // 2) * P : (c // 2 + 1) * P],
                            rhs=qt2[half : half + H, q_sl],
                            start=True,
                            stop=True,
                            tile_position=(half, 0),
                        )
                    pt = ptp.tile([P, 2 * TQ], MMD, tag="pt", name=f"pt{j}_{pr}")
                    nc.scalar.activation(
                        out=pt,
                        in_=pss,
                        func=mybir.ActivationFunctionType.Exp,
                        scale=SCALE,
                    )
                    for h2 in range(2):
                        c = 2 * pr + h2
                        d = c - SUB * j
                        if d >= 0:
                            nc.vector.tensor_mul(
                                out=pt[:, h2 * TQ : (h2 + 1) * TQ],
                                in0=pt[:, h2 * TQ : (h2 + 1) * TQ],
                                in1=masks[:, d, :],
                            )
                    for h2 in range(2):
                        c = 2 * pr + h2
                        nc.tensor.matmul(
                            pso,
                            lhsT=vext[:, c, 0 : H + 1],
                            rhs=pt[:, h2 * TQ : (h2 + 1) * TQ],
                            start=(c == 0),
                            stop=(c == nchunks - 1),
                        )
                    pump(pr + 1, npairs)
                ot = otp.tile([VW, TQ], MMD, tag="ot", name=f"ot{j}")
                nc.vector.tensor_copy(out=ot[0 : H + 1, :], in_=pso)
                # O^T -> O via the DMA XBAR: [80, 512] -> [128, 4, 80]
                o16 = op_.tile([P, SUB, VW], MMD, tag="o16", name=f"o16{j}")
                nc.sync.dma_start_transpose(out=o16, in_=ot)
                rv = rvp.tile([P, SUB], F32, tag="rv", name=f"rv{j}")
                nc.vector.reciprocal(out=rv, in_=o16[:, :, H : H + 1])
                o = op_.tile([P, SUB, H], F32, tag="o", name=f"o{j}")
                for i in range(SUB):
                    nc.vector.tensor_scalar_mul(
                        out=o[:, i, :],
                        in0=o16[:, i, 0:H],
                        scalar1=rv[:, i : i + 1],
                    )
                nc.gpsimd.dma_start(
                    out=out_re[:, SUB * j : SUB * (j + 1), :], in_=o
                )

            for j in range(min(2, NQ)):
                for _ in phase1_gen(j):
                    pass
            for j in range(NQ):
                gen = phase1_gen(j + 2) if j + 2 < NQ else None
                adv = {"n": 0}

                def pump(done, total, gen=gen, adv=adv):
                    if gen is None:
                        return
                    want = done * N1_CHUNKS // total
                    while adv["n"] < want:
                        try:
                            next(gen)
                        except StopIteration:
                            break
                        adv["n"] += 1

                phase2(j, pump)
                if gen is not None:
                    for _ in gen:
                        pass

    nc.compile()
    return nc


_NC_CACHE = {}


def _get_nc():
    if "nc" not in _NC_CACHE:
        _NC_CACHE["nc"] = build_nc()
    return _NC_CACHE["nc"]


def kernel(x, Wk, Wq, Wv, _trace=False, _trace_kwargs=None):
    x = np.ascontiguousarray(x, dtype=np.float32)
    Wk = np.ascontiguousarray(Wk, dtype=np.float32)
    Wq = np.ascontiguousarray(Wq, dtype=np.float32)
    Wv = np.ascontiguousarray(Wv, dtype=np.float32)
    nc = _get_nc()
    in_maps = [
        {"x": x[b], "Wq": Wq, "Wk": Wk, "Wv": Wv} for b in range(B)
    ]
    res = run_bass_kernel_spmd(
        nc, in_maps, list(range(B)), trace=_trace, **(_trace_kwargs or {})
    )
    out = np.stack([res.results[b]["out"] for b in range(B)], axis=0)
    if _trace:
        return out, res
    return out
